# revision 10
# baseline (speedup 1.0000x reference)
"""Trainium2 Bass kernel for a GQA attention block (B=2, S=2048, H=2048,
16 q-heads / 8 kv-heads, head_dim=128, fp32), tensor-parallel over heads
across 8 NeuronCores.

Per-core shard (core c): q-heads {2c, 2c+1}, kv-head c; wq/wk/wv column
shards, wo row shard. x is replicated (pre-transposed on host so the
contraction dim lands on SBUF partitions). Each core emits a partial
[4096, 2048] o-proj product; the host gather for the row-parallel o-proj
is a sum over the 8 partials.

Device dataflow (per core):
  A) QKV^T projections ([d, tok] layout) via float32r matmuls; one ACT
     copy evicts each PSUM head slab to SBUF; RMSNorm sum-of-squares via
     GPSIMD partition-allreduce (the q/k norm weights are folded into the
     RoPE tables on the host); RoPE as partition-half shuffle; the rstd
     scale is applied after RoPE (commutes -- rstd is column-uniform).
     V is transposed back to natural [tok, d] via PE transposes.
  B) Causal attention, two sub-phases per (batch, q-tile, head):
     (1) S^T tiles [128 k, 512 q] = K^T_tile.T @ Q^T + exp on ACT (no max
         subtraction -- RMSNorm bounds |scores| <= sqrt(128)) + causal
         affine_select on the diagonal band;
     (2) softmax denominator (ones-vector matmuls) and PV (V_nat as
         stationary) accumulated over k-tiles.
     Then the row-parallel o-proj partial, streamed out per 512-row tile.
"""

import math
import os
import sys

import numpy as np

for _p in ("/opt/trn_rl_repo", "/root/.axon_site/_ro/trn_rl_repo"):
    if os.path.isdir(_p) and _p not in sys.path:
        sys.path.insert(0, _p)
        break

import concourse.bacc as bacc
import concourse.tile as tile
from concourse import mybir
from concourse.bass_isa import ReduceOp
from concourse.bass_utils import run_bass_kernel_spmd
from concourse.masks import make_identity

# Problem constants (hardcoded per contract)
B, S, HID = 2, 2048, 2048
NH, NKV, D = 16, 8, 128
NCORES = 8
HQ = NH // NCORES  # q heads per core = 2
T = B * S          # 4096 tokens
EPS = 1e-5
F32 = mybir.dt.float32
F32R = mybir.dt.float32r
SCALE = 1.0 / math.sqrt(D)

KT = HID // 128      # 16 contraction tiles
TT = T // 512        # 8 token tiles of 512
QT_PER_B = S // 512  # 4 q-tiles per batch


def build_nc():
    nc = bacc.Bacc("TRN2", target_bir_lowering=False, debug=False)
    xt = nc.dram_tensor("xt", [HID, T], F32R, kind="ExternalInput").ap()
    wqkv = nc.dram_tensor("wqkv", [HID, 4 * D], F32R, kind="ExternalInput").ap()
    woc = nc.dram_tensor("woc", [HQ * D, HID], F32R, kind="ExternalInput").ap()
    pmat = nc.dram_tensor("pmat", [D, D], F32R, kind="ExternalInput").ap()
    onec = nc.dram_tensor("onec", [D, 1], F32R, kind="ExternalInput").ap()
    ctq = nc.dram_tensor("ctq", [D, S], F32, kind="ExternalInput").ap()
    stq = nc.dram_tensor("stq", [D, S], F32, kind="ExternalInput").ap()
    ctk = nc.dram_tensor("ctk", [D, S], F32, kind="ExternalInput").ap()
    stk = nc.dram_tensor("stk", [D, S], F32, kind="ExternalInput").ap()
    out = nc.dram_tensor("out", [T, HID], F32, kind="ExternalOutput").ap()

    with tile.TileContext(nc) as tc:
        from contextlib import ExitStack

        with ExitStack() as root:
            const = root.enter_context(tc.tile_pool(name="const", bufs=1))
            ident = const.tile([128, 128], F32, name="ident")
            make_identity(nc, ident)
            ones_col = const.tile([128, 1], F32R, name="ones_col")
            nc.sync.dma_start(out=ones_col, in_=onec)
            pmat_sb = const.tile([D, D], F32R, name="pmat_sb")
            nc.sync.dma_start(out=pmat_sb, in_=pmat)
            eps_col = const.tile([128, 1], F32, name="eps_col")
            nc.vector.memset(eps_col, EPS)

            res = root.enter_context(tc.tile_pool(name="res", bufs=1))
            wo_sb = res.tile([128, HQ, HID], F32R, name="wo_sb")
            qt_sb = res.tile([128, HQ, T], F32R, name="qt_sb")   # [d, h, tok]
            kt_sb = res.tile([128, T], F32R, name="kt_sb")       # [d, tok]
            v_sb = res.tile([128, T // 128, D], F32R, name="v_sb")  # [tok%128, tile, d]

            # ---------------- Phase A: QKV^T, norm, rope, V transpose ---------
            with ExitStack() as pa:
                wqp = pa.enter_context(tc.tile_pool(name="wqp", bufs=1))
                xp = pa.enter_context(tc.tile_pool(name="xp", bufs=17))
                tabp = pa.enter_context(tc.tile_pool(name="tabp", bufs=2))
                wp = pa.enter_context(tc.tile_pool(name="wp", bufs=2))
                psA = pa.enter_context(tc.tile_pool(name="psA", bufs=2, space="PSUM"))
                psT = pa.enter_context(tc.tile_pool(name="psT", bufs=2, space="PSUM"))
                psR = pa.enter_context(tc.tile_pool(name="psR", bufs=2, space="PSUM"))

                wqkv_sb = wqp.tile([128, KT, 4 * D], F32R, name="wqkv_sb")

                for t in range(TT):
                    xks = []
                    for k in range(KT):
                        if t == 0:  # interleave weight loads with first x tiles
                            nc.sync.dma_start(
                                out=wqkv_sb[:, k, :], in_=wqkv[k * 128:(k + 1) * 128, :]
                            )
                        xk = xp.tile([128, 512], F32R, name="xk", tag="xk")
                        nc.sync.dma_start(
                            out=xk, in_=xt[k * 128:(k + 1) * 128, t * 512:(t + 1) * 512]
                        )
                        xks.append(xk)
                    if t == 1:  # wo is not needed until phase B
                        nc.sync.dma_start(
                            out=wo_sb, in_=woc.rearrange("(h p) n -> p h n", p=128)
                        )
                    # two 2-bank PSUM slabs: (q0,q1) and (k,v)
                    slabs = []
                    for g in range(2):
                        ps = psA.tile([128, 2, 512], F32, name="ps_qkv", tag="ps_qkv")
                        for k in range(KT):
                            for mm in range(2):
                                m = g * 2 + mm
                                nc.tensor.matmul(
                                    ps[:, mm, :],
                                    lhsT=(wqkv_sb[:, k, m * 128:(m + 1) * 128]),
                                    rhs=(xks[k]),
                                    start=(k == 0),
                                    stop=(k == KT - 1),
                                )
                        slabs.append(ps)

                    s0 = (t % QT_PER_B) * 512  # position-in-sequence of this tile
                    tabs = {}
                    for nm, ap in (("cq", ctq), ("sq", stq), ("ck", ctk), ("sk", stk)):
                        tl = tabp.tile([128, 512], F32, name="tab_" + nm, tag="tab_" + nm)
                        nc.sync.dma_start(out=tl, in_=ap[:, s0:s0 + 512])
                        tabs[nm] = tl
                    for m, cosT, sinT in (
                        (0, tabs["cq"], tabs["sq"]),
                        (1, tabs["cq"], tabs["sq"]),
                        (2, tabs["ck"], tabs["sk"]),
                    ):
                        src = slabs[m // 2][:, m % 2, :]
                        qk = wp.tile([128, 512], F32R, name="qk", tag="qk")
                        nc.scalar.copy(qk, src)  # sole PSUM reader (ACT)
                        sq = wp.tile([128, 512], F32, name="sq", tag="sq")
                        nc.vector.tensor_mul(sq, qk, qk)
                        nc.gpsimd.partition_all_reduce(sq, sq, 128, ReduceOp.add)
                        rrow = wp.tile([1, 512], F32, name="rrow", tag="rrow")
                        nc.scalar.activation(
                            rrow, sq[0:1, :], mybir.ActivationFunctionType.Sqrt,
                            bias=eps_col[0:1, :], scale=1.0 / D,
                        )
                        nc.vector.reciprocal(rrow, rrow)
                        rstd = wp.tile([128, 512], F32, name="rstd", tag="rstd")
                        nc.gpsimd.partition_broadcast(rstd, rrow)
                        shf = psR.tile([128, 512], F32, name="shf", tag="shf")
                        nc.tensor.matmul(shf, lhsT=pmat_sb, rhs=qk, start=True, stop=True)
                        t0 = wp.tile([128, 512], F32, name="t0", tag="t0")
                        nc.vector.tensor_mul(t0, qk, cosT)
                        t1 = wp.tile([128, 512], F32, name="t1", tag="t1")
                        nc.vector.tensor_mul(t1, shf, sinT)
                        tr = wp.tile([128, 512], F32, name="tr", tag="tr")
                        nc.vector.tensor_add(tr, t0, t1)
                        if m < 2:
                            dst = qt_sb[:, m, t * 512:(t + 1) * 512]
                        else:
                            dst = kt_sb[:, t * 512:(t + 1) * 512]
                        nc.vector.tensor_mul(dst, tr, rstd)
                    # V: evict transposed VT then PE-transpose to natural
                    vt = wp.tile([128, 512], F32, name="vt", tag="vt")
                    nc.scalar.copy(vt, slabs[1][:, 1, :])
                    for j in range(4):
                        pv = psT.tile([128, 128], F32, name="pv", tag="pv")
                        nc.tensor.transpose(pv, vt[:, j * 128:(j + 1) * 128], ident)
                        nc.scalar.copy(v_sb[:, t * 4 + j, :], pv)

            # ---------------- Phase B: causal attention + o-proj --------------
            with ExitStack() as pb:
                ep = pb.enter_context(tc.tile_pool(name="ep", bufs=18))
                wp2 = pb.enter_context(tc.tile_pool(name="wp2", bufs=3))
                atp = pb.enter_context(tc.tile_pool(name="atp", bufs=4))
                op = pb.enter_context(tc.tile_pool(name="op", bufs=4))
                psS = pb.enter_context(tc.tile_pool(name="psS", bufs=3, space="PSUM"))
                psO = pb.enter_context(tc.tile_pool(name="psO", bufs=2, space="PSUM"))
                psD = pb.enter_context(tc.tile_pool(name="psD", bufs=1, space="PSUM"))
                psP = pb.enter_context(tc.tile_pool(name="psP", bufs=2, space="PSUM"))

                for b in range(B):
                    for qt in range(QT_PER_B):
                        q0 = qt * 512
                        n_kt = 4 * (qt + 1)
                        at_tiles = []
                        for h in range(HQ):
                            # sub-phase 1: scores + exp (+ causal mask)
                            ets = []
                            for kt in range(n_kt):
                                st = psS.tile([128, 512], F32, name="st", tag="st")
                                nc.tensor.matmul(
                                    st,
                                    lhsT=(kt_sb[:, b * S + kt * 128: b * S + (kt + 1) * 128]),
                                    rhs=(qt_sb[:, h, b * S + q0: b * S + q0 + 512]),
                                    start=True, stop=True,
                                )
                                et = ep.tile([128, 512], F32R, name="et", tag="et")
                                nc.scalar.activation(
                                    et, st, mybir.ActivationFunctionType.Exp,
                                    scale=SCALE,
                                )
                                if kt >= 4 * qt:  # diagonal band: causal mask
                                    nc.gpsimd.affine_select(
                                        out=et, in_=et,
                                        pattern=[[1, 512]],
                                        channel_multiplier=-1,
                                        base=-(kt * 128 - q0),
                                        compare_op=mybir.AluOpType.is_ge,
                                        fill=0.0,
                                    )
                                ets.append(et)
                            # sub-phase 2: denominator + PV accumulation
                            ot = psO.tile([128, 512], F32, name="ot", tag="ot")
                            den = psD.tile([1, 512], F32, name="den", tag="den")
                            for kt in range(n_kt):
                                nc.tensor.matmul(
                                    den, lhsT=ones_col, rhs=ets[kt],
                                    start=(kt == 0), stop=(kt == n_kt - 1),
                                )
                                nc.tensor.matmul(
                                    ot, lhsT=(v_sb[:, b * (S // 128) + kt, :]),
                                    rhs=(ets[kt]),
                                    start=(kt == 0), stop=(kt == n_kt - 1),
                                )
                            rd = wp2.tile([1, 512], F32, name="rd", tag="rd")
                            nc.vector.reciprocal(rd, den)
                            rb = wp2.tile([128, 512], F32, name="rb", tag="rb")
                            nc.gpsimd.partition_broadcast(rb, rd)
                            at = atp.tile([128, 512], F32R, name="at", tag="at")
                            nc.vector.tensor_mul(at, ot, rb)
                            at_tiles.append(at)
                        # o-proj partial for rows [b*S+q0, +512)
                        for mq in range(4):
                            for nn in range(4):
                                po = psP.tile([128, 512], F32, name="po", tag="po")
                                for h in range(HQ):
                                    nc.tensor.matmul(
                                        po,
                                        lhsT=(at_tiles[h][:, mq * 128:(mq + 1) * 128]),
                                        rhs=(wo_sb[:, h, nn * 512:(nn + 1) * 512]),
                                        start=(h == 0), stop=(h == HQ - 1),
                                    )
                                ob = op.tile([128, 512], F32, name="ob", tag="ob")
                                if (mq + nn) % 2 == 0:
                                    nc.vector.tensor_copy(ob, po)
                                else:
                                    nc.scalar.copy(ob, po)
                                nc.sync.dma_start(
                                    out=out[b * S + q0 + mq * 128: b * S + q0 + (mq + 1) * 128,
                                            nn * 512:(nn + 1) * 512],
                                    in_=ob,
                                )
    nc.compile()
    return nc


def _rot_half(w):
    return np.concatenate([w[D // 2:], w[:D // 2]])


def prep_inputs(x, cos, sin, wq, wk, wv, wo, q_norm_w, k_norm_w):
    """Host-side sharding/layout prep. Returns per-core in_maps."""
    f = np.float32
    x = np.asarray(x, f)
    cos = np.asarray(cos, f)
    sin = np.asarray(sin, f)
    wq, wk, wv, wo = (np.asarray(a, f) for a in (wq, wk, wv, wo))
    q_norm_w = np.asarray(q_norm_w, f)
    k_norm_w = np.asarray(k_norm_w, f)

    xt = np.ascontiguousarray(x.reshape(T, HID).T)  # [HID, T]
    ctq = np.ascontiguousarray(cos.T * q_norm_w[:, None])
    stq = np.ascontiguousarray(sin.T * _rot_half(q_norm_w)[:, None])
    ctk = np.ascontiguousarray(cos.T * k_norm_w[:, None])
    stk = np.ascontiguousarray(sin.T * _rot_half(k_norm_w)[:, None])
    # rotate-half permutation (with sign) as a matmul stationary operand:
    # out[d] = sum_j pmat[j, d] * q[j] = sign(d) * q[(d+64) % 128]
    pmat = np.zeros((D, D), f)
    for d in range(D // 2):
        pmat[d + D // 2, d] = -1.0
    for d in range(D // 2, D):
        pmat[d - D // 2, d] = 1.0
    onec = np.ones((D, 1), f)

    in_maps = []
    for c in range(NCORES):
        wqkv_c = np.ascontiguousarray(np.concatenate([
            wq[:, c * HQ * D:(c + 1) * HQ * D],
            wk[:, c * D:(c + 1) * D],
            wv[:, c * D:(c + 1) * D],
        ], axis=1))
        woc = np.ascontiguousarray(wo[c * HQ * D:(c + 1) * HQ * D, :])
        in_maps.append({
            "xt": xt, "wqkv": wqkv_c, "woc": woc, "pmat": pmat, "onec": onec,
            "ctq": ctq, "stq": stq, "ctk": ctk, "stk": stk,
        })
    return in_maps


_NC = None


def get_nc():
    global _NC
    if _NC is None:
        _NC = build_nc()
    return _NC


def kernel(x, cos, sin, wq, wk, wv, wo, q_norm_w, k_norm_w):
    nc = get_nc()
    in_maps = prep_inputs(x, cos, sin, wq, wk, wv, wo, q_norm_w, k_norm_w)
    res = run_bass_kernel_spmd(nc, in_maps, core_ids=list(range(NCORES)))
    acc = np.zeros((T, HID), dtype=np.float64)
    for c in range(NCORES):
        acc += res.results[c]["out"]
    return acc.astype(np.float32).reshape(B, S, HID)


# revision 23
# speedup vs baseline: 328.8964x; 328.8964x over previous
"""Trainium2 Bass kernel for a GQA attention block (B=2, S=2048, H=2048,
16 q-heads / 8 kv-heads, head_dim=128, fp32), tensor-parallel over heads
across 8 NeuronCores.

Per-core shard (core c): q-heads {2c, 2c+1}, kv-head c; wq/wk/wv column
shards, wo row shard. x is replicated (pre-transposed on host so the
contraction dim lands on SBUF partitions). Each core emits a partial
[4096, 2048] o-proj product; the host gather for the row-parallel o-proj
is a sum over the 8 partials.

Device dataflow (per core):
  A) QKV^T projections ([d, tok] layout) via float32r matmuls; one ACT
     copy evicts each PSUM head slab to SBUF; RMSNorm sum-of-squares via
     GPSIMD partition-allreduce (the q/k norm weights are folded into the
     RoPE tables on the host); RoPE as partition-half shuffle; the rstd
     scale is applied after RoPE (commutes -- rstd is column-uniform).
     V is transposed back to natural [tok, d] via PE transposes.
  B) Causal attention, two sub-phases per (batch, q-tile, head):
     (1) S^T tiles [128 k, 512 q] = K^T_tile.T @ Q^T + exp on ACT (no max
         subtraction -- RMSNorm bounds |scores| <= sqrt(128)) + causal
         affine_select on the diagonal band;
     (2) softmax denominator (ones-vector matmuls) and PV (V_nat as
         stationary) accumulated over k-tiles.
     Then the row-parallel o-proj partial, streamed out per 512-row tile.
"""

import math
import os
import sys

import numpy as np

for _p in ("/opt/trn_rl_repo", "/root/.axon_site/_ro/trn_rl_repo"):
    if os.path.isdir(_p) and _p not in sys.path:
        sys.path.insert(0, _p)
        break

import concourse.bacc as bacc
import concourse.tile as tile
from concourse import mybir
from concourse.bass_isa import ReduceOp
from concourse.bass_utils import run_bass_kernel_spmd
from concourse.masks import make_identity

# Problem constants (hardcoded per contract)
B, S, HID = 2, 2048, 2048
NH, NKV, D = 16, 8, 128
NCORES = 8
HQ = NH // NCORES  # q heads per core = 2
T = B * S          # 4096 tokens
EPS = 1e-5
F32 = mybir.dt.float32
F32R = mybir.dt.float32r
BF16 = mybir.dt.bfloat16
# matmul input dtype: "f32r" (near-fp32, default) or "bf16" (halves phase-A
# DMA; ~1e-3-class output error)
KDT = os.environ.get("BASS_KDT", "f32r")
MDT = BF16 if KDT == "bf16" else F32R
NP_MDT = None  # set lazily in prep_inputs (ml_dtypes import)
# transpose path (identity matmul) dtype: f32r can't be memset/ldweights'd,
# so use plain f32 there in f32r mode
TDT = BF16 if KDT == "bf16" else F32
SCALE = 1.0 / math.sqrt(D)

KT = HID // 128      # 16 contraction tiles
TT = T // 512        # 8 token tiles of 512
QT_PER_B = S // 512  # 4 q-tiles per batch


def build_nc():
    nc = bacc.Bacc("TRN2", target_bir_lowering=False, debug=False)
    xt = nc.dram_tensor("xt", [HID, T], MDT, kind="ExternalInput").ap()
    wqkv = nc.dram_tensor("wqkv", [HID, 4 * D], MDT, kind="ExternalInput").ap()
    woc = nc.dram_tensor("woc", [HQ * D, HID], MDT, kind="ExternalInput").ap()
    pmat = nc.dram_tensor("pmat", [D, D], MDT, kind="ExternalInput").ap()
    onec = nc.dram_tensor("onec", [D, 1], MDT, kind="ExternalInput").ap()
    ctq = nc.dram_tensor("ctq", [D, S], MDT, kind="ExternalInput").ap()
    stq = nc.dram_tensor("stq", [D, S], MDT, kind="ExternalInput").ap()
    ctk = nc.dram_tensor("ctk", [D, S], MDT, kind="ExternalInput").ap()
    stk = nc.dram_tensor("stk", [D, S], MDT, kind="ExternalInput").ap()
    out = nc.dram_tensor("out", [T, HID], F32, kind="ExternalOutput").ap()

    with tile.TileContext(nc) as tc:
        from contextlib import ExitStack

        with ExitStack() as root:
            const = root.enter_context(tc.tile_pool(name="const", bufs=1))
            ident = const.tile([128, 128], TDT, name="ident")
            make_identity(nc, ident)
            ones_col = const.tile([128, 1], MDT, name="ones_col")
            nc.scalar.dma_start(out=ones_col, in_=onec)
            pmat_sb = const.tile([D, D], MDT, name="pmat_sb")
            nc.scalar.dma_start(out=pmat_sb, in_=pmat)
            eps_col = const.tile([128, 1], F32, name="eps_col")
            nc.vector.memset(eps_col, EPS)

            res = root.enter_context(tc.tile_pool(name="res", bufs=1))
            wo_sb = res.tile([128, HQ, HID], MDT, name="wo_sb")
            qt_sb = res.tile([128, HQ, T], MDT, name="qt_sb")   # [d, h, tok]
            kt_sb = res.tile([128, T], MDT, name="kt_sb")       # [d, tok]
            v_sb = res.tile([128, T // 128, D], MDT, name="v_sb")  # [tok%128, tile, d]

            # ---------------- Phase A: QKV^T, norm, rope, V transpose ---------
            with ExitStack() as pa:
                wqp = pa.enter_context(tc.tile_pool(name="wqp", bufs=1))
                xp = pa.enter_context(tc.tile_pool(name="xp", bufs=17))
                tabp = pa.enter_context(tc.tile_pool(name="tabp", bufs=2))
                wp = pa.enter_context(tc.tile_pool(name="wp", bufs=2))
                psA = pa.enter_context(tc.tile_pool(name="psA", bufs=2, space="PSUM"))
                psT = pa.enter_context(tc.tile_pool(name="psT", bufs=2, space="PSUM"))
                psR = pa.enter_context(tc.tile_pool(name="psR", bufs=2, space="PSUM"))

                wqkv_sb = wqp.tile([128, KT, 4 * D], MDT, name="wqkv_sb")

                for t in range(TT):
                    xks = []
                    for k in range(KT):
                        if t == 0:  # interleave weight loads with first x tiles
                            nc.sync.dma_start(
                                out=wqkv_sb[:, k, :], in_=wqkv[k * 128:(k + 1) * 128, :]
                            )
                        xk = xp.tile([128, 512], MDT, name="xk", tag="xk")
                        nc.sync.dma_start(
                            out=xk, in_=xt[k * 128:(k + 1) * 128, t * 512:(t + 1) * 512]
                        )
                        xks.append(xk)
                    if t == 1:  # wo is not needed until phase B
                        nc.sync.dma_start(
                            out=wo_sb, in_=woc.rearrange("(h p) n -> p h n", p=128)
                        )
                    # two 2-bank PSUM slabs: (q0,q1) and (k,v)
                    slabs = []
                    for g in range(2):
                        ps = psA.tile([128, 2, 512], F32, name="ps_qkv", tag="ps_qkv")
                        for k in range(KT):
                            for mm in range(2):
                                m = g * 2 + mm
                                nc.tensor.matmul(
                                    ps[:, mm, :],
                                    lhsT=(wqkv_sb[:, k, m * 128:(m + 1) * 128]),
                                    rhs=(xks[k]),
                                    start=(k == 0),
                                    stop=(k == KT - 1),
                                )
                        slabs.append(ps)

                    s0 = (t % QT_PER_B) * 512  # position-in-sequence of this tile
                    tabs = {}
                    for nm, ap in (("cq", ctq), ("sq", stq), ("ck", ctk), ("sk", stk)):
                        tl = tabp.tile([128, 512], MDT, name="tab_" + nm, tag="tab_" + nm)
                        nc.sync.dma_start(out=tl, in_=ap[:, s0:s0 + 512])
                        tabs[nm] = tl
                    for m, cosT, sinT in (
                        (0, tabs["cq"], tabs["sq"]),
                        (1, tabs["cq"], tabs["sq"]),
                        (2, tabs["ck"], tabs["sk"]),
                    ):
                        src = slabs[m // 2][:, m % 2, :]
                        qk = wp.tile([128, 512], MDT, name="qk", tag="qk")
                        nc.scalar.copy(qk, src)  # sole PSUM reader (ACT)
                        sq = wp.tile([128, 512], F32, name="sq", tag="sq")
                        nc.vector.tensor_mul(sq, qk, qk)
                        nc.gpsimd.partition_all_reduce(sq, sq, 128, ReduceOp.add)
                        rrow = wp.tile([1, 512], F32, name="rrow", tag="rrow")
                        nc.scalar.activation(
                            rrow, sq[0:1, :], mybir.ActivationFunctionType.Sqrt,
                            bias=eps_col[0:1, :], scale=1.0 / D,
                        )
                        nc.vector.reciprocal(rrow, rrow)
                        rstd = wp.tile([128, 512], F32, name="rstd", tag="rstd")
                        nc.gpsimd.partition_broadcast(rstd, rrow)
                        shf = psR.tile([128, 512], F32, name="shf", tag="shf")
                        nc.tensor.matmul(shf, lhsT=pmat_sb, rhs=qk, start=True, stop=True)
                        t0 = wp.tile([128, 512], F32, name="t0", tag="t0")
                        nc.vector.tensor_mul(t0, qk, cosT)
                        t1 = wp.tile([128, 512], F32, name="t1", tag="t1")
                        nc.vector.tensor_mul(t1, shf, sinT)
                        tr = wp.tile([128, 512], F32, name="tr", tag="tr")
                        nc.vector.tensor_add(tr, t0, t1)
                        if m < 2:
                            dst = qt_sb[:, m, t * 512:(t + 1) * 512]
                        else:
                            dst = kt_sb[:, t * 512:(t + 1) * 512]
                        nc.vector.tensor_mul(dst, tr, rstd)
                    # V: evict transposed VT then PE-transpose to natural
                    vt = wp.tile([128, 512], TDT, name="vt", tag="vt")
                    nc.scalar.copy(vt, slabs[1][:, 1, :])
                    for j in range(4):
                        pv = psT.tile([128, 128], TDT, name="pv", tag="pv")
                        nc.tensor.transpose(pv, vt[:, j * 128:(j + 1) * 128], ident)
                        nc.scalar.copy(v_sb[:, t * 4 + j, :], pv)

            # ---------------- Phase B: causal attention + o-proj --------------
            with ExitStack() as pb:
                ep = pb.enter_context(tc.tile_pool(name="ep", bufs=20))
                wp2 = pb.enter_context(tc.tile_pool(name="wp2", bufs=3))
                atp = pb.enter_context(tc.tile_pool(name="atp", bufs=8))
                op = pb.enter_context(tc.tile_pool(name="op", bufs=4))
                psS = pb.enter_context(tc.tile_pool(name="psS", bufs=3, space="PSUM"))
                psO = pb.enter_context(tc.tile_pool(name="psO", bufs=2, space="PSUM"))
                psD = pb.enter_context(tc.tile_pool(name="psD", bufs=1, space="PSUM"))
                psP = pb.enter_context(tc.tile_pool(name="psP", bufs=2, space="PSUM"))

                for b in range(B):
                    for qt in range(QT_PER_B):
                        q0 = qt * 512
                        at_tiles = {}
                        for h in range(HQ):
                            for qh in range(2):  # 256-wide q slices
                                qq0 = q0 + qh * 256
                                n_kt = (qq0 + 256) // 128  # valid k tiles
                                # sub-phase 1: scores, two k-tiles packed
                                # per PSUM bank, one exp per pair, causal mask
                                ets = [None] * n_kt
                                for kp in range(n_kt // 2):
                                    st = psS.tile([128, 2, 256], F32, name="st", tag="st")
                                    for j in range(2):
                                        kt = 2 * kp + j
                                        nc.tensor.matmul(
                                            st[:, j, :],
                                            lhsT=(kt_sb[:, b * S + kt * 128: b * S + (kt + 1) * 128]),
                                            rhs=(qt_sb[:, h, b * S + qq0: b * S + qq0 + 256]),
                                            start=True, stop=True,
                                        )
                                    etp = ep.tile([128, 2, 256], MDT, name="et", tag="et")
                                    nc.scalar.activation(
                                        etp, st, mybir.ActivationFunctionType.Exp,
                                        scale=SCALE,
                                    )
                                    for j in range(2):
                                        kt = 2 * kp + j
                                        et = etp[:, j, :]
                                        if kt * 128 + 127 > qq0:  # diagonal band
                                            nc.gpsimd.affine_select(
                                                out=et, in_=et,
                                                pattern=[[1, 256]],
                                                channel_multiplier=-1,
                                                base=-(kt * 128 - qq0),
                                                compare_op=mybir.AluOpType.is_ge,
                                                fill=0.0,
                                            )
                                        ets[kt] = et
                                # sub-phase 2: denominator + PV accumulation
                                ot = psO.tile([128, 256], F32, name="ot", tag="ot")
                                den = psD.tile([1, 256], F32, name="den", tag="den")
                                for kt in range(n_kt):
                                    nc.tensor.matmul(
                                        den, lhsT=ones_col, rhs=ets[kt],
                                        start=(kt == 0), stop=(kt == n_kt - 1),
                                    )
                                    nc.tensor.matmul(
                                        ot, lhsT=(v_sb[:, b * (S // 128) + kt, :]),
                                        rhs=(ets[kt]),
                                        start=(kt == 0), stop=(kt == n_kt - 1),
                                    )
                                rd = wp2.tile([1, 256], F32, name="rd", tag="rd")
                                nc.vector.reciprocal(rd, den)
                                rb = wp2.tile([128, 256], F32, name="rb", tag="rb")
                                nc.gpsimd.partition_broadcast(rb, rd)
                                at = atp.tile([128, 256], MDT, name="at", tag="at")
                                nc.vector.tensor_mul(at, ot, rb)
                                at_tiles[(h, qh)] = at
                        # o-proj partial for rows [b*S+q0, +512)
                        for mq in range(4):
                            qh = mq // 2
                            mq2 = mq % 2  # 128-slice within the 256 at tile
                            for nn in range(4):
                                po = psP.tile([128, 512], F32, name="po", tag="po")
                                for h in range(HQ):
                                    nc.tensor.matmul(
                                        po,
                                        lhsT=(at_tiles[(h, qh)][:, mq2 * 128:(mq2 + 1) * 128]),
                                        rhs=(wo_sb[:, h, nn * 512:(nn + 1) * 512]),
                                        start=(h == 0), stop=(h == HQ - 1),
                                    )
                                ob = op.tile([128, 512], F32, name="ob", tag="ob")
                                nc.vector.tensor_copy(ob, po)
                                nc.sync.dma_start(
                                    out=out[b * S + q0 + mq * 128: b * S + q0 + (mq + 1) * 128,
                                            nn * 512:(nn + 1) * 512],
                                    in_=ob,
                                )
    nc.compile()
    return nc


def _rot_half(w):
    return np.concatenate([w[D // 2:], w[:D // 2]])


def prep_inputs(x, cos, sin, wq, wk, wv, wo, q_norm_w, k_norm_w):
    """Host-side sharding/layout prep. Returns per-core in_maps."""
    f = np.float32
    if KDT == "bf16":
        import ml_dtypes
        mf = np.dtype(ml_dtypes.bfloat16)
    else:
        mf = np.float32
    cvt = lambda a: np.ascontiguousarray(a.astype(mf))
    x = np.asarray(x, f)
    cos = np.asarray(cos, f)
    sin = np.asarray(sin, f)
    wq, wk, wv, wo = (np.asarray(a, f) for a in (wq, wk, wv, wo))
    q_norm_w = np.asarray(q_norm_w, f)
    k_norm_w = np.asarray(k_norm_w, f)

    xt = np.ascontiguousarray(x.reshape(T, HID).T)  # [HID, T]
    ctq = np.ascontiguousarray(cos.T * q_norm_w[:, None])
    stq = np.ascontiguousarray(sin.T * _rot_half(q_norm_w)[:, None])
    ctk = np.ascontiguousarray(cos.T * k_norm_w[:, None])
    stk = np.ascontiguousarray(sin.T * _rot_half(k_norm_w)[:, None])
    # rotate-half permutation (with sign) as a matmul stationary operand:
    # out[d] = sum_j pmat[j, d] * q[j] = sign(d) * q[(d+64) % 128]
    pmat = np.zeros((D, D), f)
    for d in range(D // 2):
        pmat[d + D // 2, d] = -1.0
    for d in range(D // 2, D):
        pmat[d - D // 2, d] = 1.0
    onec = np.ones((D, 1), f)
    xt_m, ctq_m, stq_m, ctk_m, stk_m, pmat_m, onec_m = (
        cvt(a) for a in (xt, ctq, stq, ctk, stk, pmat, onec))

    in_maps = []
    for c in range(NCORES):
        wqkv_c = np.ascontiguousarray(np.concatenate([
            wq[:, c * HQ * D:(c + 1) * HQ * D],
            wk[:, c * D:(c + 1) * D],
            wv[:, c * D:(c + 1) * D],
        ], axis=1))
        woc = np.ascontiguousarray(wo[c * HQ * D:(c + 1) * HQ * D, :])
        in_maps.append({
            "xt": xt_m, "wqkv": cvt(wqkv_c), "woc": cvt(woc),
            "pmat": pmat_m, "onec": onec_m,
            "ctq": ctq_m, "stq": stq_m, "ctk": ctk_m, "stk": stk_m,
        })
    return in_maps


_NC = None


def get_nc():
    global _NC
    if _NC is None:
        _NC = build_nc()
    return _NC


def kernel(x, cos, sin, wq, wk, wv, wo, q_norm_w, k_norm_w):
    nc = get_nc()
    in_maps = prep_inputs(x, cos, sin, wq, wk, wv, wo, q_norm_w, k_norm_w)
    res = run_bass_kernel_spmd(nc, in_maps, core_ids=list(range(NCORES)))
    acc = np.zeros((T, HID), dtype=np.float64)
    for c in range(NCORES):
        acc += res.results[c]["out"]
    return acc.astype(np.float32).reshape(B, S, HID)


# revision 26
# speedup vs baseline: 339.0951x; 1.0310x over previous
"""Trainium2 Bass kernel for a GQA attention block (B=2, S=2048, H=2048,
16 q-heads / 8 kv-heads, head_dim=128, fp32), tensor-parallel over heads
across 8 NeuronCores.

Per-core shard (core c): q-heads {2c, 2c+1}, kv-head c; wq/wk/wv column
shards, wo row shard. x is replicated (pre-transposed on host so the
contraction dim lands on SBUF partitions). Each core emits a partial
[4096, 2048] o-proj product; the host gather for the row-parallel o-proj
is a sum over the 8 partials.

Device dataflow (per core):
  A) QKV^T projections ([d, tok] layout) via float32r matmuls; one ACT
     copy evicts each PSUM head slab to SBUF; RMSNorm sum-of-squares via
     GPSIMD partition-allreduce (the q/k norm weights are folded into the
     RoPE tables on the host); RoPE as partition-half shuffle; the rstd
     scale is applied after RoPE (commutes -- rstd is column-uniform).
     V is transposed back to natural [tok, d] via PE transposes.
  B) Causal attention, two sub-phases per (batch, q-tile, head):
     (1) S^T tiles [128 k, 512 q] = K^T_tile.T @ Q^T + exp on ACT (no max
         subtraction -- RMSNorm bounds |scores| <= sqrt(128)) + causal
         affine_select on the diagonal band;
     (2) softmax denominator (ones-vector matmuls) and PV (V_nat as
         stationary) accumulated over k-tiles.
     Then the row-parallel o-proj partial, streamed out per 512-row tile.
"""

import math
import os
import sys

import numpy as np

for _p in ("/opt/trn_rl_repo", "/root/.axon_site/_ro/trn_rl_repo"):
    if os.path.isdir(_p) and _p not in sys.path:
        sys.path.insert(0, _p)
        break

import concourse.bacc as bacc
import concourse.tile as tile
from concourse import mybir
from concourse.bass_isa import ReduceOp
from concourse.bass_utils import run_bass_kernel_spmd
from concourse.masks import make_identity

# Problem constants (hardcoded per contract)
B, S, HID = 2, 2048, 2048
NH, NKV, D = 16, 8, 128
NCORES = 8
HQ = NH // NCORES  # q heads per core = 2
T = B * S          # 4096 tokens
EPS = 1e-5
F32 = mybir.dt.float32
F32R = mybir.dt.float32r
BF16 = mybir.dt.bfloat16
# matmul input dtype: "f32r" (near-fp32, default) or "bf16" (halves phase-A
# DMA; ~1e-3-class output error)
KDT = os.environ.get("BASS_KDT", "f32r")
MDT = BF16 if KDT == "bf16" else F32R
NP_MDT = None  # set lazily in prep_inputs (ml_dtypes import)
# transpose path (identity matmul) dtype: f32r can't be memset/ldweights'd,
# so use plain f32 there in f32r mode
TDT = BF16 if KDT == "bf16" else F32
SCALE = 1.0 / math.sqrt(D)

KT = HID // 128      # 16 contraction tiles
TT = T // 512        # 8 token tiles of 512
QT_PER_B = S // 512  # 4 q-tiles per batch


def build_nc():
    nc = bacc.Bacc("TRN2", target_bir_lowering=False, debug=False)
    xt = nc.dram_tensor("xt", [HID, T], MDT, kind="ExternalInput").ap()
    wqkv = nc.dram_tensor("wqkv", [HID, 4 * D], MDT, kind="ExternalInput").ap()
    woc = nc.dram_tensor("woc", [HQ * D, HID], MDT, kind="ExternalInput").ap()
    pmat = nc.dram_tensor("pmat", [D, D], MDT, kind="ExternalInput").ap()
    onec = nc.dram_tensor("onec", [D, 1], MDT, kind="ExternalInput").ap()
    ctq = nc.dram_tensor("ctq", [D, S], MDT, kind="ExternalInput").ap()
    stq = nc.dram_tensor("stq", [D, S], MDT, kind="ExternalInput").ap()
    ctk = nc.dram_tensor("ctk", [D, S], MDT, kind="ExternalInput").ap()
    stk = nc.dram_tensor("stk", [D, S], MDT, kind="ExternalInput").ap()
    out = nc.dram_tensor("out", [T, HID], F32, kind="ExternalOutput").ap()

    with tile.TileContext(nc) as tc:
        from contextlib import ExitStack

        with ExitStack() as root:
            const = root.enter_context(tc.tile_pool(name="const", bufs=1))
            ident = const.tile([128, 128], TDT, name="ident")
            make_identity(nc, ident)
            ones_col = const.tile([128, 1], MDT, name="ones_col")
            nc.scalar.dma_start(out=ones_col, in_=onec)
            pmat_sb = const.tile([D, D], MDT, name="pmat_sb")
            nc.scalar.dma_start(out=pmat_sb, in_=pmat)
            eps_col = const.tile([128, 1], F32, name="eps_col")
            nc.vector.memset(eps_col, EPS)

            res = root.enter_context(tc.tile_pool(name="res", bufs=1))
            wo_sb = res.tile([128, HQ, HID], MDT, name="wo_sb")
            qt_sb = res.tile([128, HQ, T], MDT, name="qt_sb")   # [d, h, tok]
            kt_sb = res.tile([128, T], MDT, name="kt_sb")       # [d, tok]
            v_sb = res.tile([128, T // 128, D], MDT, name="v_sb")  # [tok%128, tile, d]

            # ---------------- Phase A: QKV^T, norm, rope, V transpose ---------
            with ExitStack() as pa:
                wqp = pa.enter_context(tc.tile_pool(name="wqp", bufs=1))
                xp = pa.enter_context(tc.tile_pool(name="xp", bufs=17))
                tabp = pa.enter_context(tc.tile_pool(name="tabp", bufs=2))
                wp = pa.enter_context(tc.tile_pool(name="wp", bufs=2))
                psA = pa.enter_context(tc.tile_pool(name="psA", bufs=2, space="PSUM"))
                psT = pa.enter_context(tc.tile_pool(name="psT", bufs=2, space="PSUM"))
                psR = pa.enter_context(tc.tile_pool(name="psR", bufs=2, space="PSUM"))

                wqkv_sb = wqp.tile([128, KT, 4 * D], MDT, name="wqkv_sb")

                # visit token tiles as (b0, b1) pairs sharing a sequence
                # position so each RoPE table slice is fetched once
                tabs = {}
                for ti, t in enumerate((0, 4, 1, 5, 2, 6, 3, 7)):
                    xks = []
                    for k in range(KT):
                        if ti == 0:  # interleave weight loads with first x tiles
                            nc.sync.dma_start(
                                out=wqkv_sb[:, k, :], in_=wqkv[k * 128:(k + 1) * 128, :]
                            )
                        xk = xp.tile([128, 512], MDT, name="xk", tag="xk")
                        nc.sync.dma_start(
                            out=xk, in_=xt[k * 128:(k + 1) * 128, t * 512:(t + 1) * 512]
                        )
                        xks.append(xk)
                    if ti == 5:  # wo is not needed until phase B
                        nc.sync.dma_start(
                            out=wo_sb, in_=woc.rearrange("(h p) n -> p h n", p=128)
                        )
                    # two 2-bank PSUM slabs: (q0,q1) and (k,v)
                    slabs = []
                    for g in range(2):
                        ps = psA.tile([128, 2, 512], F32, name="ps_qkv", tag="ps_qkv")
                        for k in range(KT):
                            for mm in range(2):
                                m = g * 2 + mm
                                nc.tensor.matmul(
                                    ps[:, mm, :],
                                    lhsT=(wqkv_sb[:, k, m * 128:(m + 1) * 128]),
                                    rhs=(xks[k]),
                                    start=(k == 0),
                                    stop=(k == KT - 1),
                                )
                        slabs.append(ps)

                    s0 = (t % QT_PER_B) * 512  # position-in-sequence of this tile
                    if ti % 2 == 0:  # second tile of each pair reuses the slices
                        tabs = {}
                        for nm, ap in (("cq", ctq), ("sq", stq), ("ck", ctk), ("sk", stk)):
                            tl = tabp.tile([128, 512], MDT, name="tab_" + nm, tag="tab_" + nm)
                            nc.sync.dma_start(out=tl, in_=ap[:, s0:s0 + 512])
                            tabs[nm] = tl
                    for m, cosT, sinT in (
                        (0, tabs["cq"], tabs["sq"]),
                        (1, tabs["cq"], tabs["sq"]),
                        (2, tabs["ck"], tabs["sk"]),
                    ):
                        src = slabs[m // 2][:, m % 2, :]
                        qk = wp.tile([128, 512], MDT, name="qk", tag="qk")
                        nc.scalar.copy(qk, src)  # sole PSUM reader (ACT)
                        sq = wp.tile([128, 512], F32, name="sq", tag="sq")
                        nc.vector.tensor_mul(sq, qk, qk)
                        nc.gpsimd.partition_all_reduce(sq, sq, 128, ReduceOp.add)
                        rrow = wp.tile([1, 512], F32, name="rrow", tag="rrow")
                        nc.scalar.activation(
                            rrow, sq[0:1, :], mybir.ActivationFunctionType.Sqrt,
                            bias=eps_col[0:1, :], scale=1.0 / D,
                        )
                        nc.vector.reciprocal(rrow, rrow)
                        rstd = wp.tile([128, 512], F32, name="rstd", tag="rstd")
                        nc.gpsimd.partition_broadcast(rstd, rrow)
                        shf = psR.tile([128, 512], F32, name="shf", tag="shf")
                        nc.tensor.matmul(shf, lhsT=pmat_sb, rhs=qk, start=True, stop=True)
                        t0 = wp.tile([128, 512], F32, name="t0", tag="t0")
                        nc.vector.tensor_mul(t0, qk, cosT)
                        t1 = wp.tile([128, 512], F32, name="t1", tag="t1")
                        nc.vector.tensor_mul(t1, shf, sinT)
                        tr = wp.tile([128, 512], F32, name="tr", tag="tr")
                        nc.vector.tensor_add(tr, t0, t1)
                        if m < 2:
                            dst = qt_sb[:, m, t * 512:(t + 1) * 512]
                        else:
                            dst = kt_sb[:, t * 512:(t + 1) * 512]
                        nc.vector.tensor_mul(dst, tr, rstd)
                    # V: evict transposed VT then PE-transpose to natural
                    vt = wp.tile([128, 512], TDT, name="vt", tag="vt")
                    nc.scalar.copy(vt, slabs[1][:, 1, :])
                    for j in range(4):
                        pv = psT.tile([128, 128], TDT, name="pv", tag="pv")
                        nc.tensor.transpose(pv, vt[:, j * 128:(j + 1) * 128], ident)
                        nc.scalar.copy(v_sb[:, t * 4 + j, :], pv)

            # ---------------- Phase B: causal attention + o-proj --------------
            with ExitStack() as pb:
                ep = pb.enter_context(tc.tile_pool(name="ep", bufs=20))
                wp2 = pb.enter_context(tc.tile_pool(name="wp2", bufs=3))
                atp = pb.enter_context(tc.tile_pool(name="atp", bufs=8))
                op = pb.enter_context(tc.tile_pool(name="op", bufs=4))
                psS = pb.enter_context(tc.tile_pool(name="psS", bufs=3, space="PSUM"))
                psO = pb.enter_context(tc.tile_pool(name="psO", bufs=2, space="PSUM"))
                psD = pb.enter_context(tc.tile_pool(name="psD", bufs=1, space="PSUM"))
                psP = pb.enter_context(tc.tile_pool(name="psP", bufs=2, space="PSUM"))

                for b in range(B):
                    for qt in range(QT_PER_B):
                        q0 = qt * 512
                        at_tiles = {}
                        for h in range(HQ):
                            for qh in range(2):  # 256-wide q slices
                                qq0 = q0 + qh * 256
                                n_kt = (qq0 + 256) // 128  # valid k tiles
                                # sub-phase 1: scores, two k-tiles packed
                                # per PSUM bank, one exp per pair, causal mask
                                ets = [None] * n_kt
                                for kp in range(n_kt // 2):
                                    st = psS.tile([128, 2, 256], F32, name="st", tag="st")
                                    for j in range(2):
                                        kt = 2 * kp + j
                                        nc.tensor.matmul(
                                            st[:, j, :],
                                            lhsT=(kt_sb[:, b * S + kt * 128: b * S + (kt + 1) * 128]),
                                            rhs=(qt_sb[:, h, b * S + qq0: b * S + qq0 + 256]),
                                            start=True, stop=True,
                                        )
                                    etp = ep.tile([128, 2, 256], MDT, name="et", tag="et")
                                    nc.scalar.activation(
                                        etp, st, mybir.ActivationFunctionType.Exp,
                                        scale=SCALE,
                                    )
                                    for j in range(2):
                                        kt = 2 * kp + j
                                        et = etp[:, j, :]
                                        if kt * 128 + 127 > qq0:  # diagonal band
                                            nc.gpsimd.affine_select(
                                                out=et, in_=et,
                                                pattern=[[1, 256]],
                                                channel_multiplier=-1,
                                                base=-(kt * 128 - qq0),
                                                compare_op=mybir.AluOpType.is_ge,
                                                fill=0.0,
                                            )
                                        ets[kt] = et
                                # sub-phase 2: denominator + PV accumulation
                                ot = psO.tile([128, 256], F32, name="ot", tag="ot")
                                den = psD.tile([1, 256], F32, name="den", tag="den")
                                for kt in range(n_kt):
                                    nc.tensor.matmul(
                                        den, lhsT=ones_col, rhs=ets[kt],
                                        start=(kt == 0), stop=(kt == n_kt - 1),
                                    )
                                    nc.tensor.matmul(
                                        ot, lhsT=(v_sb[:, b * (S // 128) + kt, :]),
                                        rhs=(ets[kt]),
                                        start=(kt == 0), stop=(kt == n_kt - 1),
                                    )
                                rd = wp2.tile([1, 256], F32, name="rd", tag="rd")
                                nc.vector.reciprocal(rd, den)
                                rb = wp2.tile([128, 256], F32, name="rb", tag="rb")
                                nc.gpsimd.partition_broadcast(rb, rd)
                                at = atp.tile([128, 256], MDT, name="at", tag="at")
                                nc.vector.tensor_mul(at, ot, rb)
                                at_tiles[(h, qh)] = at
                        # o-proj partial for rows [b*S+q0, +512)
                        for mq in range(4):
                            qh = mq // 2
                            mq2 = mq % 2  # 128-slice within the 256 at tile
                            for nn in range(4):
                                po = psP.tile([128, 512], F32, name="po", tag="po")
                                for h in range(HQ):
                                    nc.tensor.matmul(
                                        po,
                                        lhsT=(at_tiles[(h, qh)][:, mq2 * 128:(mq2 + 1) * 128]),
                                        rhs=(wo_sb[:, h, nn * 512:(nn + 1) * 512]),
                                        start=(h == 0), stop=(h == HQ - 1),
                                    )
                                ob = op.tile([128, 512], F32, name="ob", tag="ob")
                                nc.vector.tensor_copy(ob, po)
                                nc.sync.dma_start(
                                    out=out[b * S + q0 + mq * 128: b * S + q0 + (mq + 1) * 128,
                                            nn * 512:(nn + 1) * 512],
                                    in_=ob,
                                )
    nc.compile()
    return nc


def _rot_half(w):
    return np.concatenate([w[D // 2:], w[:D // 2]])


def prep_inputs(x, cos, sin, wq, wk, wv, wo, q_norm_w, k_norm_w):
    """Host-side sharding/layout prep. Returns per-core in_maps."""
    f = np.float32
    if KDT == "bf16":
        import ml_dtypes
        mf = np.dtype(ml_dtypes.bfloat16)
    else:
        mf = np.float32
    cvt = lambda a: np.ascontiguousarray(a.astype(mf))
    x = np.asarray(x, f)
    cos = np.asarray(cos, f)
    sin = np.asarray(sin, f)
    wq, wk, wv, wo = (np.asarray(a, f) for a in (wq, wk, wv, wo))
    q_norm_w = np.asarray(q_norm_w, f)
    k_norm_w = np.asarray(k_norm_w, f)

    xt = np.ascontiguousarray(x.reshape(T, HID).T)  # [HID, T]
    ctq = np.ascontiguousarray(cos.T * q_norm_w[:, None])
    stq = np.ascontiguousarray(sin.T * _rot_half(q_norm_w)[:, None])
    ctk = np.ascontiguousarray(cos.T * k_norm_w[:, None])
    stk = np.ascontiguousarray(sin.T * _rot_half(k_norm_w)[:, None])
    # rotate-half permutation (with sign) as a matmul stationary operand:
    # out[d] = sum_j pmat[j, d] * q[j] = sign(d) * q[(d+64) % 128]
    pmat = np.zeros((D, D), f)
    for d in range(D // 2):
        pmat[d + D // 2, d] = -1.0
    for d in range(D // 2, D):
        pmat[d - D // 2, d] = 1.0
    onec = np.ones((D, 1), f)
    xt_m, ctq_m, stq_m, ctk_m, stk_m, pmat_m, onec_m = (
        cvt(a) for a in (xt, ctq, stq, ctk, stk, pmat, onec))

    in_maps = []
    for c in range(NCORES):
        wqkv_c = np.ascontiguousarray(np.concatenate([
            wq[:, c * HQ * D:(c + 1) * HQ * D],
            wk[:, c * D:(c + 1) * D],
            wv[:, c * D:(c + 1) * D],
        ], axis=1))
        woc = np.ascontiguousarray(wo[c * HQ * D:(c + 1) * HQ * D, :])
        in_maps.append({
            "xt": xt_m, "wqkv": cvt(wqkv_c), "woc": cvt(woc),
            "pmat": pmat_m, "onec": onec_m,
            "ctq": ctq_m, "stq": stq_m, "ctk": ctk_m, "stk": stk_m,
        })
    return in_maps


_NC = None


def get_nc():
    global _NC
    if _NC is None:
        _NC = build_nc()
    return _NC


def kernel(x, cos, sin, wq, wk, wv, wo, q_norm_w, k_norm_w):
    nc = get_nc()
    in_maps = prep_inputs(x, cos, sin, wq, wk, wv, wo, q_norm_w, k_norm_w)
    res = run_bass_kernel_spmd(nc, in_maps, core_ids=list(range(NCORES)))
    acc = np.zeros((T, HID), dtype=np.float64)
    for c in range(NCORES):
        acc += res.results[c]["out"]
    return acc.astype(np.float32).reshape(B, S, HID)


# revision 27
# speedup vs baseline: 339.5761x; 1.0014x over previous
"""Trainium2 Bass kernel for a GQA attention block (B=2, S=2048, H=2048,
16 q-heads / 8 kv-heads, head_dim=128, fp32), tensor-parallel over heads
across 8 NeuronCores.

Per-core shard (core c): q-heads {2c, 2c+1}, kv-head c; wq/wk/wv column
shards, wo row shard. x is replicated (pre-transposed on host so the
contraction dim lands on SBUF partitions). Each core emits a partial
[4096, 2048] o-proj product; the host gather for the row-parallel o-proj
is a sum over the 8 partials.

Device dataflow (per core):
  A) QKV^T projections ([d, tok] layout) via float32r matmuls; one ACT
     copy evicts each PSUM head slab to SBUF; RMSNorm sum-of-squares via
     GPSIMD partition-allreduce (the q/k norm weights are folded into the
     RoPE tables on the host); RoPE as partition-half shuffle; the rstd
     scale is applied after RoPE (commutes -- rstd is column-uniform).
     V is transposed back to natural [tok, d] via PE transposes.
  B) Causal attention, two sub-phases per (batch, q-tile, head):
     (1) S^T tiles [128 k, 512 q] = K^T_tile.T @ Q^T + exp on ACT (no max
         subtraction -- RMSNorm bounds |scores| <= sqrt(128)) + causal
         affine_select on the diagonal band;
     (2) softmax denominator (ones-vector matmuls) and PV (V_nat as
         stationary) accumulated over k-tiles.
     Then the row-parallel o-proj partial, streamed out per 512-row tile.
"""

import math
import os
import sys

import numpy as np

for _p in ("/opt/trn_rl_repo", "/root/.axon_site/_ro/trn_rl_repo"):
    if os.path.isdir(_p) and _p not in sys.path:
        sys.path.insert(0, _p)
        break

import concourse.bacc as bacc
import concourse.tile as tile
from concourse import mybir
from concourse.bass_isa import ReduceOp
from concourse.bass_utils import run_bass_kernel_spmd
from concourse.masks import make_identity

# Problem constants (hardcoded per contract)
B, S, HID = 2, 2048, 2048
NH, NKV, D = 16, 8, 128
NCORES = 8
HQ = NH // NCORES  # q heads per core = 2
T = B * S          # 4096 tokens
EPS = 1e-5
F32 = mybir.dt.float32
F32R = mybir.dt.float32r
BF16 = mybir.dt.bfloat16
# matmul input dtype: "f32r" (near-fp32, default) or "bf16" (halves phase-A
# DMA; ~1e-3-class output error)
KDT = os.environ.get("BASS_KDT", "f32r")
MDT = BF16 if KDT == "bf16" else F32R
NP_MDT = None  # set lazily in prep_inputs (ml_dtypes import)
# transpose path (identity matmul) dtype: f32r can't be memset/ldweights'd,
# so use plain f32 there in f32r mode
TDT = BF16 if KDT == "bf16" else F32
SCALE = 1.0 / math.sqrt(D)

KT = HID // 128      # 16 contraction tiles
TT = T // 512        # 8 token tiles of 512
QT_PER_B = S // 512  # 4 q-tiles per batch


def build_nc():
    nc = bacc.Bacc("TRN2", target_bir_lowering=False, debug=False)
    xt = nc.dram_tensor("xt", [HID, T], MDT, kind="ExternalInput").ap()
    wqkv = nc.dram_tensor("wqkv", [HID, 4 * D], MDT, kind="ExternalInput").ap()
    woc = nc.dram_tensor("woc", [HQ * D, HID], MDT, kind="ExternalInput").ap()
    pmat = nc.dram_tensor("pmat", [D, D], MDT, kind="ExternalInput").ap()
    onec = nc.dram_tensor("onec", [D, 1], MDT, kind="ExternalInput").ap()
    ctq = nc.dram_tensor("ctq", [D, S], MDT, kind="ExternalInput").ap()
    stq = nc.dram_tensor("stq", [D, S], MDT, kind="ExternalInput").ap()
    ctk = nc.dram_tensor("ctk", [D, S], MDT, kind="ExternalInput").ap()
    stk = nc.dram_tensor("stk", [D, S], MDT, kind="ExternalInput").ap()
    out = nc.dram_tensor("out", [T, HID], F32, kind="ExternalOutput").ap()

    with tile.TileContext(nc) as tc:
        from contextlib import ExitStack

        with ExitStack() as root:
            const = root.enter_context(tc.tile_pool(name="const", bufs=1))
            ident = const.tile([128, 128], TDT, name="ident")
            make_identity(nc, ident)
            ones_col = const.tile([128, 1], MDT, name="ones_col")
            nc.scalar.dma_start(out=ones_col, in_=onec)
            pmat_sb = const.tile([D, D], MDT, name="pmat_sb")
            nc.scalar.dma_start(out=pmat_sb, in_=pmat)
            eps_col = const.tile([128, 1], F32, name="eps_col")
            nc.vector.memset(eps_col, EPS)

            res = root.enter_context(tc.tile_pool(name="res", bufs=1))
            wo_sb = res.tile([128, HQ, HID], MDT, name="wo_sb")
            qt_sb = res.tile([128, HQ, T], MDT, name="qt_sb")   # [d, h, tok]
            kt_sb = res.tile([128, T], MDT, name="kt_sb")       # [d, tok]
            v_sb = res.tile([128, T // 128, D], MDT, name="v_sb")  # [tok%128, tile, d]

            # ---------------- Phase A: QKV^T, norm, rope, V transpose ---------
            with ExitStack() as pa:
                wqp = pa.enter_context(tc.tile_pool(name="wqp", bufs=1))
                xp = pa.enter_context(tc.tile_pool(name="xp", bufs=17))
                tabp = pa.enter_context(tc.tile_pool(name="tabp", bufs=2))
                wp = pa.enter_context(tc.tile_pool(name="wp", bufs=2))
                psA = pa.enter_context(tc.tile_pool(name="psA", bufs=2, space="PSUM"))
                psT = pa.enter_context(tc.tile_pool(name="psT", bufs=2, space="PSUM"))
                psR = pa.enter_context(tc.tile_pool(name="psR", bufs=2, space="PSUM"))

                wqkv_sb = wqp.tile([128, KT, 4 * D], MDT, name="wqkv_sb")

                # visit token tiles as (b0, b1) pairs sharing a sequence
                # position so each RoPE table slice is fetched once
                tabs = {}
                for ti, t in enumerate((0, 4, 1, 5, 2, 6, 3, 7)):
                    xks = []
                    for k in range(KT):
                        if ti == 0:  # interleave weight loads with first x tiles
                            nc.sync.dma_start(
                                out=wqkv_sb[:, k, :], in_=wqkv[k * 128:(k + 1) * 128, :]
                            )
                        xk = xp.tile([128, 512], MDT, name="xk", tag="xk")
                        nc.sync.dma_start(
                            out=xk, in_=xt[k * 128:(k + 1) * 128, t * 512:(t + 1) * 512]
                        )
                        xks.append(xk)
                    if ti == 5:  # wo is not needed until phase B
                        nc.sync.dma_start(
                            out=wo_sb, in_=woc.rearrange("(h p) n -> p h n", p=128)
                        )
                    # two 2-bank PSUM slabs: (q0,q1) and (k,v)
                    slabs = []
                    for g in range(2):
                        ps = psA.tile([128, 2, 512], F32, name="ps_qkv", tag="ps_qkv")
                        for k in range(KT):
                            for mm in range(2):
                                m = g * 2 + mm
                                nc.tensor.matmul(
                                    ps[:, mm, :],
                                    lhsT=(wqkv_sb[:, k, m * 128:(m + 1) * 128]),
                                    rhs=(xks[k]),
                                    start=(k == 0),
                                    stop=(k == KT - 1),
                                )
                        slabs.append(ps)

                    s0 = (t % QT_PER_B) * 512  # position-in-sequence of this tile
                    if ti % 2 == 0:  # second tile of each pair reuses the slices
                        tabs = {}
                        for nm, ap in (("cq", ctq), ("sq", stq), ("ck", ctk), ("sk", stk)):
                            tl = tabp.tile([128, 512], MDT, name="tab_" + nm, tag="tab_" + nm)
                            nc.sync.dma_start(out=tl, in_=ap[:, s0:s0 + 512])
                            tabs[nm] = tl
                    for m, cosT, sinT in (
                        (0, tabs["cq"], tabs["sq"]),
                        (1, tabs["cq"], tabs["sq"]),
                        (2, tabs["ck"], tabs["sk"]),
                    ):
                        src = slabs[m // 2][:, m % 2, :]
                        qk = wp.tile([128, 512], MDT, name="qk", tag="qk")
                        nc.scalar.copy(qk, src)  # sole PSUM reader (ACT)
                        sq = wp.tile([128, 512], F32, name="sq", tag="sq")
                        nc.vector.tensor_mul(sq, qk, qk)
                        nc.gpsimd.partition_all_reduce(sq, sq, 128, ReduceOp.add)
                        rrow = wp.tile([1, 512], F32, name="rrow", tag="rrow")
                        nc.scalar.activation(
                            rrow, sq[0:1, :], mybir.ActivationFunctionType.Sqrt,
                            bias=eps_col[0:1, :], scale=1.0 / D,
                        )
                        nc.vector.reciprocal(rrow, rrow)
                        rstd = wp.tile([128, 512], F32, name="rstd", tag="rstd")
                        nc.gpsimd.partition_broadcast(rstd, rrow)
                        shf = psR.tile([128, 512], F32, name="shf", tag="shf")
                        nc.tensor.matmul(shf, lhsT=pmat_sb, rhs=qk, start=True, stop=True)
                        t0 = wp.tile([128, 512], F32, name="t0", tag="t0")
                        nc.vector.tensor_mul(t0, qk, cosT)
                        t1 = wp.tile([128, 512], F32, name="t1", tag="t1")
                        nc.vector.tensor_mul(t1, shf, sinT)
                        tr = wp.tile([128, 512], F32, name="tr", tag="tr")
                        nc.vector.tensor_add(tr, t0, t1)
                        if m < 2:
                            dst = qt_sb[:, m, t * 512:(t + 1) * 512]
                        else:
                            dst = kt_sb[:, t * 512:(t + 1) * 512]
                        nc.vector.tensor_mul(dst, tr, rstd)
                    # V: evict transposed VT then PE-transpose to natural
                    vt = wp.tile([128, 512], TDT, name="vt", tag="vt")
                    nc.scalar.copy(vt, slabs[1][:, 1, :])
                    for j in range(4):
                        pv = psT.tile([128, 128], TDT, name="pv", tag="pv")
                        nc.tensor.transpose(pv, vt[:, j * 128:(j + 1) * 128], ident)
                        nc.scalar.copy(v_sb[:, t * 4 + j, :], pv)

            # ---------------- Phase B: causal attention + o-proj --------------
            with ExitStack() as pb:
                ep = pb.enter_context(tc.tile_pool(name="ep", bufs=20))
                wp2 = pb.enter_context(tc.tile_pool(name="wp2", bufs=3))
                atp = pb.enter_context(tc.tile_pool(name="atp", bufs=8))
                op = pb.enter_context(tc.tile_pool(name="op", bufs=4))
                psS = pb.enter_context(tc.tile_pool(name="psS", bufs=3, space="PSUM"))
                psO = pb.enter_context(tc.tile_pool(name="psO", bufs=2, space="PSUM"))
                psD = pb.enter_context(tc.tile_pool(name="psD", bufs=1, space="PSUM"))
                psP = pb.enter_context(tc.tile_pool(name="psP", bufs=2, space="PSUM"))

                for b in range(B):
                    for qt in range(QT_PER_B):
                        q0 = qt * 512
                        at_tiles = {}
                        for h in range(HQ):
                            for qh in range(2):  # 256-wide q slices
                                qq0 = q0 + qh * 256
                                n_kt = (qq0 + 256) // 128  # valid k tiles
                                # sub-phase 1: scores, two k-tiles packed
                                # per PSUM bank, one exp per pair, causal mask
                                ets = [None] * n_kt
                                for kp in range(n_kt // 2):
                                    st = psS.tile([128, 2, 256], F32, name="st", tag="st")
                                    for j in range(2):
                                        kt = 2 * kp + j
                                        nc.tensor.matmul(
                                            st[:, j, :],
                                            lhsT=(kt_sb[:, b * S + kt * 128: b * S + (kt + 1) * 128]),
                                            rhs=(qt_sb[:, h, b * S + qq0: b * S + qq0 + 256]),
                                            start=True, stop=True,
                                        )
                                    etp = ep.tile([128, 2, 256], MDT, name="et", tag="et")
                                    nc.scalar.activation(
                                        etp, st, mybir.ActivationFunctionType.Exp,
                                        scale=SCALE,
                                    )
                                    for j in range(2):
                                        kt = 2 * kp + j
                                        et = etp[:, j, :]
                                        if kt * 128 + 127 > qq0:  # diagonal band
                                            nc.gpsimd.affine_select(
                                                out=et, in_=et,
                                                pattern=[[1, 256]],
                                                channel_multiplier=-1,
                                                base=-(kt * 128 - qq0),
                                                compare_op=mybir.AluOpType.is_ge,
                                                fill=0.0,
                                            )
                                        ets[kt] = et
                                # sub-phase 2: denominator + PV accumulation
                                ot = psO.tile([128, 256], F32, name="ot", tag="ot")
                                den = psD.tile([1, 256], F32, name="den", tag="den")
                                for kt in range(n_kt):
                                    nc.tensor.matmul(
                                        den, lhsT=ones_col, rhs=ets[kt],
                                        start=(kt == 0), stop=(kt == n_kt - 1),
                                    )
                                    nc.tensor.matmul(
                                        ot, lhsT=(v_sb[:, b * (S // 128) + kt, :]),
                                        rhs=(ets[kt]),
                                        start=(kt == 0), stop=(kt == n_kt - 1),
                                    )
                                rd = wp2.tile([1, 256], F32, name="rd", tag="rd")
                                nc.vector.reciprocal(rd, den)
                                rb = wp2.tile([128, 256], F32, name="rb", tag="rb")
                                nc.gpsimd.partition_broadcast(rb, rd)
                                at = atp.tile([128, 256], MDT, name="at", tag="at")
                                nc.vector.tensor_mul(at, ot, rb)
                                at_tiles[(h, qh)] = at
                        # o-proj partial for rows [b*S+q0, +512)
                        for mq in range(4):
                            qh = mq // 2
                            mq2 = mq % 2  # 128-slice within the 256 at tile
                            for nn in range(4):
                                po = psP.tile([128, 512], F32, name="po", tag="po")
                                for h in range(HQ):
                                    nc.tensor.matmul(
                                        po,
                                        lhsT=(at_tiles[(h, qh)][:, mq2 * 128:(mq2 + 1) * 128]),
                                        rhs=(wo_sb[:, h, nn * 512:(nn + 1) * 512]),
                                        start=(h == 0), stop=(h == HQ - 1),
                                    )
                                ob = op.tile([128, 512], F32, name="ob", tag="ob")
                                # batch 1: ACT has slack (phase-A tail done) and
                                # DVE is the mid-phase-B choke; batch 0: keep DVE
                                if b == 1 and (mq + nn) % 2 == 0:
                                    nc.scalar.copy(ob, po)
                                else:
                                    nc.vector.tensor_copy(ob, po)
                                nc.sync.dma_start(
                                    out=out[b * S + q0 + mq * 128: b * S + q0 + (mq + 1) * 128,
                                            nn * 512:(nn + 1) * 512],
                                    in_=ob,
                                )
    nc.compile()
    return nc


def _rot_half(w):
    return np.concatenate([w[D // 2:], w[:D // 2]])


def prep_inputs(x, cos, sin, wq, wk, wv, wo, q_norm_w, k_norm_w):
    """Host-side sharding/layout prep. Returns per-core in_maps."""
    f = np.float32
    if KDT == "bf16":
        import ml_dtypes
        mf = np.dtype(ml_dtypes.bfloat16)
    else:
        mf = np.float32
    cvt = lambda a: np.ascontiguousarray(a.astype(mf))
    x = np.asarray(x, f)
    cos = np.asarray(cos, f)
    sin = np.asarray(sin, f)
    wq, wk, wv, wo = (np.asarray(a, f) for a in (wq, wk, wv, wo))
    q_norm_w = np.asarray(q_norm_w, f)
    k_norm_w = np.asarray(k_norm_w, f)

    xt = np.ascontiguousarray(x.reshape(T, HID).T)  # [HID, T]
    ctq = np.ascontiguousarray(cos.T * q_norm_w[:, None])
    stq = np.ascontiguousarray(sin.T * _rot_half(q_norm_w)[:, None])
    ctk = np.ascontiguousarray(cos.T * k_norm_w[:, None])
    stk = np.ascontiguousarray(sin.T * _rot_half(k_norm_w)[:, None])
    # rotate-half permutation (with sign) as a matmul stationary operand:
    # out[d] = sum_j pmat[j, d] * q[j] = sign(d) * q[(d+64) % 128]
    pmat = np.zeros((D, D), f)
    for d in range(D // 2):
        pmat[d + D // 2, d] = -1.0
    for d in range(D // 2, D):
        pmat[d - D // 2, d] = 1.0
    onec = np.ones((D, 1), f)
    xt_m, ctq_m, stq_m, ctk_m, stk_m, pmat_m, onec_m = (
        cvt(a) for a in (xt, ctq, stq, ctk, stk, pmat, onec))

    in_maps = []
    for c in range(NCORES):
        wqkv_c = np.ascontiguousarray(np.concatenate([
            wq[:, c * HQ * D:(c + 1) * HQ * D],
            wk[:, c * D:(c + 1) * D],
            wv[:, c * D:(c + 1) * D],
        ], axis=1))
        woc = np.ascontiguousarray(wo[c * HQ * D:(c + 1) * HQ * D, :])
        in_maps.append({
            "xt": xt_m, "wqkv": cvt(wqkv_c), "woc": cvt(woc),
            "pmat": pmat_m, "onec": onec_m,
            "ctq": ctq_m, "stq": stq_m, "ctk": ctk_m, "stk": stk_m,
        })
    return in_maps


_NC = None


def get_nc():
    global _NC
    if _NC is None:
        _NC = build_nc()
    return _NC


def kernel(x, cos, sin, wq, wk, wv, wo, q_norm_w, k_norm_w):
    nc = get_nc()
    in_maps = prep_inputs(x, cos, sin, wq, wk, wv, wo, q_norm_w, k_norm_w)
    res = run_bass_kernel_spmd(nc, in_maps, core_ids=list(range(NCORES)))
    acc = np.zeros((T, HID), dtype=np.float64)
    for c in range(NCORES):
        acc += res.results[c]["out"]
    return acc.astype(np.float32).reshape(B, S, HID)
